# revision 16
# baseline (speedup 1.0000x reference)
"""NetVLAD pooling kernel for Trainium2 (8 NeuronCores, data-parallel over B).

Math (per batch row b):
    logits = feats @ assign_w.T              # (L, K); assign_b cancels in softmax over L
    a_u    = exp(logits + maskbias)          # maskbias = -1e30 for l >= lens[b]
    U      = a_u.T @ feats                   # (K, D) unnormalized
    s      = sum_l a_u[l, :]                 # (K,)
    vlad   = U / s - centroids               # host
    out    = l2norm(vlad.min(axis=0))        # host

Device structure (per core: 4 batch rows, fully python-unrolled):
  Both feats layouts ship in fp8-e4m3 (harness gate is 2e-2 rel err; this
  measures ~9e-3), halving HBM traffic vs bf16. All matmuls run in
  DoubleRow perf mode (two 128-row contraction subtiles per instruction),
  so operand tiles are 3-D: [128, ksub, free].

  Tokens at l >= lens[b] get softmax weight exactly 0 (exp(-1e30+x) == 0),
  so whole segments past ceil(lens/seg_l) are skipped: rows are sorted by
  segment count and dealt across the 8 cores so all cores share one module
  whose per-slot trip counts (caps) cover the longest row in each slot.

  pass A: psum_lg[128L, 64K] += ft[:, 2d:2d+2, l:l+128].T @ wt[:, 2d:2d+2, :]
  exp:    ACT: a_u[:, j, :] = Exp(psum_lg + mask_col) -> fp8 SBUF
  pass B: psum_U[64K, 512] += a_u[128,2,64].T @ nat[:, 2p:2p+2, 0:512]
          psum_s[64K, 1]   += a_u.T @ ones[128,2,1]
"""

import numpy as np

import concourse.bass as bass
import concourse.mybir as mybir
import concourse.tile as tile
from concourse import bacc
from concourse.bass_utils import run_bass_kernel_spmd

B, L, D, K = 32, 4096, 1024, 64
NCORES = 8
BPC = B // NCORES          # batch rows per core
F32 = mybir.dt.float32
FP8 = mybir.dt.float8e4    # e4m3
DR = mybir.MatmulPerfMode.DoubleRow

# segmentation: L-segments per row and prefetch depth
NSEG = 16
FBUFS = 10


def build_kernel(caps, bpc=BPC, l=L, d=D, k=K, fbufs=FBUFS, nseg=NSEG):
    """Build + compile the per-core module for per-slot segment counts
    ``caps`` (len bpc). All 8 cores run this same module."""
    lt = l // 128           # L-tiles per row (32)
    dc = d // 128           # 128-deep D chunks (8)
    dp = dc // 2            # DoubleRow D-chunk pairs (4)
    seg_l = l // nseg       # tokens per segment
    spt = seg_l // 128      # L-tiles per segment
    prs = spt // 2          # L-tile pairs per segment
    assert prs >= 1 and all(1 <= c <= nseg for c in caps)
    sumc = sum(caps)
    base = [sum(caps[:j]) for j in range(bpc)]

    nc = bacc.Bacc(None, target_bir_lowering=False, debug=False)
    ft_hbm = nc.dram_tensor("ft", [sumc, 128, dc, seg_l], FP8,
                            kind="ExternalInput")
    nat_hbm = nc.dram_tensor("nat", [sumc, 128, spt, d], FP8,
                             kind="ExternalInput")
    wt_hbm = nc.dram_tensor("wt", [128, dc, k], FP8, kind="ExternalInput")
    mask_hbm = nc.dram_tensor("mask_t", [128, bpc * lt], F32, kind="ExternalInput")
    out_us = nc.dram_tensor("out_us", [bpc, k, d + 1], mybir.dt.bfloat16,
                            kind="ExternalOutput")

    with tile.TileContext(nc) as tc:
        with (
            tc.tile_pool(name="consts", bufs=1) as consts,
            tc.tile_pool(name="ft", bufs=fbufs) as ftpool,
            tc.tile_pool(name="nat", bufs=fbufs) as natpool,
            tc.tile_pool(name="au", bufs=4) as aupool,
            tc.tile_pool(name="outs", bufs=bpc) as outpool,
            tc.tile_pool(name="psL", bufs=4, space="PSUM") as psL,
            tc.tile_pool(name="psU", bufs=1, space="PSUM") as psU,
        ):
            # consts go via Pool's SWDGE so SP/HWDGE stream feats at t=0
            wt_sb = consts.tile([128, dc, k], FP8)
            nc.gpsimd.dma_start(out=wt_sb, in_=wt_hbm[:])
            mask_sb = consts.tile([128, bpc * lt], F32)
            nc.gpsimd.dma_start(out=mask_sb, in_=mask_hbm[:])
            ones = consts.tile([128, 2, 1], FP8)
            nc.vector.memset(ones, 1.0)

            for b in range(bpc):
                nsg = caps[b]
                psum_u0 = psU.tile([k, 512], F32)
                psum_u1 = psU.tile([k, 512], F32)
                psum_s = psU.tile([k, 1], F32)

                def emit_dma(sg, b=b):
                    ft = ftpool.tile([128, dc, seg_l], FP8)
                    nc.sync.dma_start(out=ft, in_=ft_hbm[base[b] + sg])
                    nat = natpool.tile([128, spt, d], FP8)
                    nc.sync.dma_start(out=nat, in_=nat_hbm[base[b] + sg])
                    return ft, nat

                def emit_passA_exp(sg, pr, ft, b=b):
                    a_u = aupool.tile([128, 2, k], FP8)
                    for jj in range(2):
                        j = pr * 2 + jj      # L-tile within segment
                        t = sg * spt + j     # global L-tile
                        psum_lg = psL.tile([128, k], F32)
                        for di2 in range(dp):
                            nc.tensor.matmul(
                                psum_lg,
                                ft[:, di2 * 2:di2 * 2 + 2, j * 128:(j + 1) * 128],
                                wt_sb[:, di2 * 2:di2 * 2 + 2, :],
                                start=(di2 == 0),
                                stop=(di2 == dp - 1),
                                perf_mode=DR,
                            )
                        nc.scalar.activation(
                            a_u[:, jj, :], psum_lg,
                            mybir.ActivationFunctionType.Exp,
                            bias=mask_sb[:, b * lt + t:b * lt + t + 1],
                        )
                    return a_u

                def emit_passB(sg, pr, a_u, nat, b=b, nsg=nsg):
                    tp = sg * prs + pr       # global pair index
                    nc.tensor.matmul(
                        psum_u0, a_u, nat[:, pr * 2:pr * 2 + 2, 0:512],
                        start=(tp == 0), stop=(tp == nsg * prs - 1),
                        perf_mode=DR,
                    )
                    nc.tensor.matmul(
                        psum_u1, a_u, nat[:, pr * 2:pr * 2 + 2, 512:1024],
                        start=(tp == 0), stop=(tp == nsg * prs - 1),
                        perf_mode=DR,
                    )
                    nc.tensor.matmul(
                        psum_s, a_u, ones,
                        start=(tp == 0), stop=(tp == nsg * prs - 1),
                        perf_mode=DR,
                    )

                # last row's final two segments are software-pipelined (pass A
                # + exp of the drain-critical last segment hoisted ahead of the
                # previous segment's pass B) so the tail chain after the final
                # nat DMA is just pass B -> copies -> out DMA
                hoist = 2 if (b == bpc - 1 and prs == 1 and nsg >= 2) else 0
                for sg in range(nsg - hoist):
                    ft, nat = emit_dma(sg)
                    for pr in range(prs):
                        a_u = emit_passA_exp(sg, pr, ft)
                        emit_passB(sg, pr, a_u, nat)
                if hoist:
                    fta, nata = emit_dma(nsg - 2)
                    ftb, natb = emit_dma(nsg - 1)
                    aua = emit_passA_exp(nsg - 2, 0, fta)
                    aub = emit_passA_exp(nsg - 1, 0, ftb)
                    emit_passB(nsg - 2, 0, aua, nata)
                    emit_passB(nsg - 1, 0, aub, natb)
                # copy U|s into one SBUF tile (DVE + ACT in parallel), then one
                # DMA out per row on the ACT queue (keeps SP's feats stream
                # free of head-of-line blocking)
                us_sb = outpool.tile([k, d + 1], mybir.dt.bfloat16)
                nc.vector.tensor_copy(us_sb[:, 0:512], psum_u0)
                nc.scalar.activation(us_sb[:, 512:1024], psum_u1,
                                     mybir.ActivationFunctionType.Copy)
                nc.vector.tensor_copy(us_sb[:, 1024:1025], psum_s)
                nc.scalar.dma_start(out=out_us[b], in_=us_sb)
    nc.compile()
    return nc


_NC_CACHE = {}
_LAST_NC = None


def _build_cached(caps):
    global _LAST_NC
    if caps not in _NC_CACHE:
        _NC_CACHE[caps] = build_kernel(caps, nseg=NSEG, fbufs=FBUFS)
    _LAST_NC = _NC_CACHE[caps]
    return _LAST_NC


def _get_nc():
    """Module of the most recent kernel() call (for timing harnesses)."""
    if _LAST_NC is None:
        # default: the segment-count pattern of the reference setup_inputs()
        _plan_shards(np.array([2078, 2141, 2218, 2412, 2467, 2507, 2676, 2699,
                               2721, 3054, 3101, 3112, 3119, 3304, 3350, 3390,
                               3444, 3517, 3517, 3525, 3640, 3681, 3741, 3746,
                               3820, 3863, 3863, 3945, 3956, 3983, 4042, 4090],
                              dtype=np.int32))
    return _LAST_NC


def _plan_shards(lens, nseg=NSEG):
    """Sort rows by live-segment count, deal across cores, build module.

    Returns (nc, perm, caps): row ``perm[8*slot + core]`` runs as slot
    ``slot`` on ``core``; ``caps[slot]`` is that slot's trip count.
    """
    seg_l = L // nseg
    counts = np.maximum(1, np.ceil(lens / seg_l).astype(int))
    perm = np.argsort(-counts, kind="stable")
    caps = tuple(int(counts[perm[NCORES * j]]) for j in range(BPC))
    nc = _build_cached(caps)
    return nc, perm, caps


def pack_host_inputs(feats, lens, assign_w, perm, caps, nseg=NSEG):
    """Host-side sharding + SBUF-order packing. Returns per-core input dicts."""
    np_f8 = mybir.dt.np(FP8)
    lt = L // 128
    dc = D // 128
    seg_l = L // nseg
    spt = seg_l // 128
    sumc = sum(caps)
    base = np.cumsum([0] + list(caps[:-1]))

    wt_host = np.ascontiguousarray(assign_w.T).reshape(dc, 128, K).astype(np_f8)
    wt_p = np.ascontiguousarray(wt_host.transpose(1, 0, 2))   # (128, dc, K)

    pos = (np.arange(lt)[None, :, None] * 128
           + np.arange(128)[None, None, :])                   # (1, lt, 128)

    in_maps = []
    for i in range(NCORES):
        rows_idx = [int(perm[NCORES * j + i]) for j in range(BPC)]
        ft_p = np.empty((sumc, 128, dc, seg_l), dtype=np_f8)
        nat_p = np.empty((sumc, 128, spt, D), dtype=np_f8)
        for j, ri in enumerate(rows_idx):
            nsg = caps[j]
            row8 = feats[ri, :nsg * seg_l].astype(np_f8)      # (nsg*seg_l, D)
            # featsT: [seg,p,di,ll] = feats[seg*seg_l+ll, di*128+p]
            ft = row8.reshape(nsg, seg_l, dc, 128).transpose(0, 3, 2, 1)
            ft_p[base[j]:base[j] + nsg] = ft
            # natural: [seg,p,jt,dd] = feats[seg*seg_l+jt*128+p, dd]
            fn = row8.reshape(nsg, spt, 128, D).transpose(0, 2, 1, 3)
            nat_p[base[j]:base[j] + nsg] = fn

        lens_core = lens[rows_idx]
        m = np.where(pos < lens_core[:, None, None], 0.0, -1e30).astype(np.float32)
        mask_t = np.ascontiguousarray(m.transpose(2, 0, 1).reshape(128, BPC * lt))

        in_maps.append({
            "ft": ft_p,
            "nat": nat_p,
            "wt": wt_p,
            "mask_t": mask_t,
        })
    return in_maps


def kernel(feats, lens, assign_w, assign_b, centroids):
    feats = np.asarray(feats, dtype=np.float32)
    lens = np.asarray(lens, dtype=np.int32)
    assign_w = np.asarray(assign_w, dtype=np.float32)
    centroids = np.asarray(centroids, dtype=np.float32)

    nc, perm, caps = _plan_shards(lens)
    in_maps = pack_host_inputs(feats, lens, assign_w, perm, caps)
    # transient device errors (NRT_EXEC_UNIT_UNRECOVERABLE) recover on retry
    last_exc = None
    for _ in range(3):
        try:
            res = run_bass_kernel_spmd(nc, in_maps, core_ids=list(range(NCORES)))
            break
        except Exception as e:  # noqa: BLE001
            last_exc = e
    else:
        raise last_exc

    out = np.empty((B, D), dtype=np.float32)
    for i in range(NCORES):
        us = np.asarray(res.results[i]["out_us"], dtype=np.float32)  # (BPC, K, D+1)
        u = us[:, :, 0:D]
        s = us[:, :, D]
        vlad = u / s[:, :, None] - centroids[None, :, :]
        o = vlad.min(axis=1)                 # (BPC, D)
        n = np.maximum(np.linalg.norm(o, axis=-1, keepdims=True), 1e-12)
        for j in range(BPC):
            out[int(perm[NCORES * j + i])] = o[j] / n[j]
    return out
